# revision 13
# baseline (speedup 1.0000x reference)
"""Trainium2 Bass kernel for a decoupled-MoE 1x1-conv container.

Math (per sample b):
    out[b] = (W_shared + weights[b] * W_routed[idx[b]]) @ x[b]
             + (b_shared + weights[b] * b_routed[idx[b]])

Strategy: data-parallel over batch B=128 across 8 NeuronCores (16 samples
per core). The expert gather/combine is a tiny O(B*C^2) host-side
computation over host-visible routing inputs; each core receives its
per-pair block-diagonal combined-weight bank [128, 8, 128] (2 samples per
128x128 lhsT tile) plus a bias bank, both precomputed on host. Activations
stream through the core in bf16 (tolerance is 2e-2; bf16 end-to-end error
is ~4e-3), which halves the HBM traffic of the memory-bound main loop:
~12.9 MB per core at ~360 GB/s. All x tiles, out tiles, and the weight
bank fit in SBUF simultaneously (~13 MB), so the DMA pipeline has no
buffer-recycle stalls: loads are issued up front and the store queue never
blocks a load.
"""

import numpy as np
import ml_dtypes

import concourse.bass as bass
import concourse.mybir as mybir
import concourse.tile as tile
from concourse.bass_utils import run_bass_kernel_spmd

F32 = mybir.dt.float32
BF16 = mybir.dt.bfloat16
NP_BF16 = ml_dtypes.bfloat16

N_CORES = 8
B = 128
C = 64  # C_IN == C_OUT == 64
H = 56
W_ = 56
HW = H * W_  # 3136
B_LOC = B // N_CORES  # 16 samples per core
PAIRS = B_LOC // 2  # 8 pairs -> [128, HW] tiles
CHUNK = 448  # 7 chunks of 448 = 3136, one PSUM bank each
N_CHUNKS = HW // CHUNK


def _legalize_waits(nc, dma_limit=1):
    """Walrus on this target allows a single sync-wait slot per engine
    compute instruction (sequencer-only instructions like InstDrain take
    many). Split excess waits onto same-engine NOPs inserted just before
    the offending instruction — semantically identical: the engine queue
    blocks on each wait in turn before executing the instruction."""
    import bass_rust

    counter = [0]
    for fn in nc.m.functions:
        for blk in fn.blocks:
            new_insts = []
            for inst in blk.instructions:
                si = inst.sync_info
                tname = type(inst).__name__
                limit = dma_limit if tname == "InstDMACopy" else 1
                if si is not None and si.on_wait and len(si.on_wait) > limit:
                    waits = list(si.on_wait)
                    keep = waits[-limit:]
                    extra = waits[:-limit]
                    for w in extra:
                        nop = mybir.InstNoOp(
                            name=f"lgl-nop-{counter[0]}", ins=[], outs=[]
                        )
                        counter[0] += 1
                        nop.engine = inst.engine
                        nop.sync_info = bass_rust.SyncInfo(
                            on_wait=[w], on_update=[]
                        )
                        new_insts.append(nop)
                    si.on_wait = keep
                new_insts.append(inst)
            blk.instructions = new_insts
    return nc


def build_program(legalize=True, nreps=1):
    nc = bass.Bass(
        "TRN2", target_bir_lowering=False, debug=False, use_seq_codegen=True
    )

    x_d = nc.dram_tensor("x", [PAIRS, 2 * C, HW], BF16, kind="ExternalInput")
    bd_d = nc.dram_tensor("bd", [2 * C, PAIRS, 2 * C], BF16, kind="ExternalInput")
    bias_d = nc.dram_tensor("bias2", [2 * C, PAIRS], F32, kind="ExternalInput")
    out_d = nc.dram_tensor("out", [PAIRS, 2 * C, HW], BF16, kind="ExternalOutput")

    with tile.TileContext(nc) as tc:
        with (
            tc.tile_pool(name="keep", bufs=1) as keep,
            tc.tile_pool(name="xs", bufs=1) as xs,
            tc.tile_pool(name="os", bufs=1) as osp,
            tc.tile_pool(name="pp", bufs=7, space="PSUM") as pp,
            tc.tile_pool(name="wp", bufs=1, space="PSUM") as wp,
        ):
            # Weight/bias banks: tiny; issued from the ACT queue so the SP
            # queue's first issue is already x0 (keeps the big-DMA stream
            # head as early as possible).
            bd = keep.tile([2 * C, PAIRS, 2 * C], BF16)
            nc.scalar.dma_start(bd, bd_d.ap())
            bias2 = keep.tile([2 * C, PAIRS], F32)
            nc.scalar.dma_start(bias2, bias_d.ap())

            # PE p-state warmup: the PE ramps (pstate low/mid) for its first
            # ~3us of activity, which would otherwise slow pair-0's chunks
            # and delay the first store (the write stream's start). Burn the
            # idle head window (~1.2us .. first-x+bd semaphore ~5.3us) with
            # dummy full-width matmuls sized to bridge INTO the first real
            # matmul so the engine arrives at it already at full clock.
            ws = keep.tile([2 * C, CHUNK], BF16)
            nc.gpsimd.memset(ws, 0.0)
            wps = wp.tile([8, CHUNK], F32)
            for _ in range(11):
                nc.tensor.matmul(
                    wps, ws[:, :8], ws, start=True, stop=True
                )

            # All x tiles persistent in SBUF: issue every load up front on
            # the SP queue; nothing ever waits to reuse these buffers.
            xts = []
            ots = []
            for pr in range(PAIRS):
                xt = xs.tile([2 * C, HW], BF16, name=f"xt{pr}")
                xts.append(xt)
                ot = osp.tile([2 * C, HW], BF16, name=f"ot{pr}")
                ots.append(ot)

            # Main loop: per pair, 7 matmul chunks (one PSUM bank each) +
            # bias epilogue alternating ACT/DVE, then the store. Stores are
            # queued on SP *after* all loads, so a store waiting on compute
            # never blocks a load. (nreps>1 repeats the full pipeline —
            # loads included — for slope-based timing.)
            # Boundary pairs stream at chunk-pair granularity: pair 0 so
            # its first store (the write stream) starts ~2 chunks after
            # the first bytes land, pair 7 so the tail chain (last load ->
            # last compute -> last store) is ~1 chunk-pair long. Subtile
            # deps let each chunk's matmul run as soon as its columns land.
            QUARTERS = [(0, 2 * CHUNK), (2 * CHUNK, 2 * CHUNK),
                        (4 * CHUNK, 2 * CHUNK), (6 * CHUNK, CHUNK)]
            HALVES = [(0, 4 * CHUNK), (4 * CHUNK, 3 * CHUNK)]
            # pair 0 at halves (quarters would starve the stream head: the
            # 650ns/issue DGE latency exceeds a quarter's transfer time);
            # pair 7 at quarters (tail: issues are long since pipelined).
            split = {0: HALVES, PAIRS - 1: QUARTERS}
            for _ in range(nreps):
                for pr in range(PAIRS):
                    if pr in split:
                        for off, w in split[pr]:
                            sl = bass.ds(off, w)
                            nc.sync.dma_start(xts[pr][:, sl], x_d[pr][:, sl])
                    else:
                        nc.sync.dma_start(xts[pr], x_d[pr])
                for pr in range(PAIRS):
                    for c in range(N_CHUNKS):
                        ps = pp.tile([2 * C, CHUNK], F32)
                        sl = bass.ds(c * CHUNK, CHUNK)
                        nc.tensor.matmul(
                            ps,
                            bd[:, pr, :],
                            xts[pr][:, sl],
                            start=True,
                            stop=True,
                        )
                        if c % 2 == 0:
                            nc.scalar.activation(
                                ots[pr][:, sl],
                                ps,
                                mybir.ActivationFunctionType.Identity,
                                bias=bias2[:, pr : pr + 1],
                            )
                        else:
                            nc.vector.tensor_scalar_add(
                                ots[pr][:, sl], ps, bias2[:, pr : pr + 1]
                            )
                    if pr in split:
                        for off, w in split[pr]:
                            sl = bass.ds(off, w)
                            nc.sync.dma_start(out_d[pr][:, sl], ots[pr][:, sl])
                    else:
                        nc.sync.dma_start(out_d[pr], ots[pr])

    if legalize:
        _legalize_waits(nc)
    return nc


_NC = None


def _get_program():
    global _NC
    if _NC is None:
        _NC = build_program()
    return _NC


def make_in_maps(x, weights, indices, W_shared, b_shared, W_routed, b_routed):
    x = np.asarray(x, dtype=np.float32)
    weights = np.asarray(weights, dtype=np.float32)
    indices = np.asarray(indices, dtype=np.int32)
    W_shared = np.asarray(W_shared, dtype=np.float32)
    b_shared = np.asarray(b_shared, dtype=np.float32)
    W_routed = np.asarray(W_routed, dtype=np.float32)
    b_routed = np.asarray(b_routed, dtype=np.float32)

    # Host-side routing: combined per-sample weights and biases (tiny).
    Wc = W_shared[None] + weights[:, None, None] * W_routed[indices]  # [B,o,i]
    bc = b_shared[None] + weights[:, None] * b_routed[indices]        # [B,o]

    xb = np.ascontiguousarray(x.reshape(B, C, HW)).astype(NP_BF16)

    in_maps = []
    for i in range(N_CORES):
        lo, hi = i * B_LOC, (i + 1) * B_LOC
        Wl = Wc[lo:hi]  # [16, o, i]
        # Block-diagonal lhsT bank bd[k, pr, m]: sample 2*pr occupies the
        # top-left 64x64 (as [i, o], transposed for lhsT), sample 2*pr+1
        # the bottom-right.
        bd = np.zeros((2 * C, PAIRS, 2 * C), dtype=np.float32)
        bd[:C, :, :C] = Wl[0::2].transpose(2, 0, 1)
        bd[C:, :, C:] = Wl[1::2].transpose(2, 0, 1)
        bl = bc[lo:hi]  # [16, o]
        bias2 = np.concatenate([bl[0::2].T, bl[1::2].T], axis=0)  # [128, 8]
        in_maps.append(
            {
                "x": xb[lo:hi].reshape(PAIRS, 2 * C, HW),
                "bd": bd.astype(NP_BF16),
                "bias2": np.ascontiguousarray(bias2, dtype=np.float32),
            }
        )
    return in_maps


def kernel(x, weights, indices, W_shared, b_shared, W_routed, b_routed):
    out, _ = _run(
        x, weights, indices, W_shared, b_shared, W_routed, b_routed, trace=False
    )
    return out


def kernel_traced(x, weights, indices, W_shared, b_shared, W_routed, b_routed):
    """Like kernel() but returns (out, BassKernelResults) with profiling."""
    return _run(
        x, weights, indices, W_shared, b_shared, W_routed, b_routed, trace=True
    )


def _run(x, weights, indices, W_shared, b_shared, W_routed, b_routed, trace):
    nc = _get_program()
    in_maps = make_in_maps(
        x, weights, indices, W_shared, b_shared, W_routed, b_routed
    )
    res = run_bass_kernel_spmd(nc, in_maps, list(range(N_CORES)), trace=trace)
    out = np.empty((B, C, H, W_), dtype=np.float32)
    for i in range(N_CORES):
        lo, hi = i * B_LOC, (i + 1) * B_LOC
        out[lo:hi] = (
            res.results[i]["out"].astype(np.float32).reshape(B_LOC, C, H, W_)
        )
    return out, res
